# revision 29
# baseline (speedup 1.0000x reference)
"""ActTransNet Trainium2 kernel (8 NeuronCores, SPMD).

Computation (per reference):
    p_avg = mean_t(precondition)            [B, 2048]
    e_avg = mean_t(effect)                  [B, 2048]
    p_embed = p_avg @ Wp.T + bp             [B, 256]
    e_embed = e_avg @ We.T + be             [B, 256]   <- output 1
    p_trans = W_trans[action] @ p_embed     [B, 256]   <- output 0

Strategy (memory-bound; everything stream-shaped is bf16, accumulation
stays f32 in PSUM, outputs round to bf16 and are upcast on host):
  - Batch is argsorted by action on the host; core i gets sorted samples
    [512*i, 512*(i+1)).  Sorting makes each core's distinct-action count
    ~B_unique/8 (~125), so each needed [256,256] table row is DMA'd once.
  - Effect path is sharded in original order (no routing needed).
  - Mean-pool over T=16 runs on the TensorEngine via a block-diagonal
    ones matrix (8 samples/tile, full 128-partition contraction); the
    1/16 is folded into the host-transposed projection weights.
  - Projection: PE matmul contracting over 2048 (pooled activations are
    PE-transposed to put the contraction dim on partitions).
  - MoE stage (transposed dataflow): host packs each per-core run of
    equal-action samples into slots of <=8 samples at fixed padded
    columns [8s, 8s+8).  p-embeddings are scattered once (4 indirect
    DMAs, host-built sample->column map) into a col-padded DRAM staging
    [ncolp, 256]; groups of 128 columns are read back, PE-transposed
    into eT [128 j, 2, ncolp] (embedding dim on partitions).  Each slot
    then runs 4 matmuls (2 j-chunks x 2 d-chunks) with its
    (host-gathered, transposed) W as the stationary operand and its 8
    embedding columns as the moving operand, accumulating into
    [128 d, 512]-column PSUM chunks.  Output leaves transposed
    [2, 128, ncolp] (d on partitions); the host transposes/de-pads.
    Padding columns carry garbage end-to-end and are never read back.
"""

import numpy as np

B, T, C, D, NA = 4096, 16, 2048, 256, 1000
M = 8            # cores
BS = B // M      # 512 samples per core
NB = BS // 128   # 4 blocks of 128 samples
SW = 8           # slot width (samples per slot, fixed column window)

_CACHE = {}

# Profiling hooks for test/dev (harness just calls kernel()).
TRACE = False
TRACE_KWARGS = {}
LAST_RES = None


def _build(nslots, reps=1, probe=False):
    import concourse.bacc as bacc
    import concourse.mybir as mybir
    import concourse.tile as tile
    from concourse import bass

    f32 = mybir.dt.float32
    bf16 = mybir.dt.bfloat16
    i32 = mybir.dt.int32
    assert nslots % 16 == 0
    ncolp = nslots * SW            # padded sample-columns, multiple of 128
    ngrp = ncolp // 128            # 128-column groups
    nchunk = (ncolp + 511) // 512  # PSUM output chunks

    nc = bacc.Bacc()
    prec = nc.declare_dram_parameter("prec", [BS * T, C], bf16, isOutput=False)
    eff = nc.declare_dram_parameter("eff", [BS * T, C], bf16, isOutput=False)
    wpt = nc.declare_dram_parameter("wpt", [C, D], bf16, isOutput=False)
    wet = nc.declare_dram_parameter("wet", [C, D], bf16, isOutput=False)
    bpp = nc.declare_dram_parameter("bp", [1, D], bf16, isOutput=False)
    bee = nc.declare_dram_parameter("be", [1, D], bf16, isOutput=False)
    ones16 = nc.declare_dram_parameter("ones16", [128, 8], bf16,
                                       isOutput=False)
    ones1 = nc.declare_dram_parameter("ones1", [1, 128], bf16, isOutput=False)
    ident = nc.declare_dram_parameter("ident", [128, 128], bf16,
                                      isOutput=False)
    # per-slot transposed expert weights: wr[s4, jp, k, jc, d]
    wr = nc.declare_dram_parameter("wr", [nslots // 4, 128, 4, 2, D], bf16,
                                   isOutput=False)
    # column destination of sample (blk*128+p) in the col-padded layout
    lidx = nc.declare_dram_parameter("lidx", [128, NB], i32, isOutput=False)
    out_pst = nc.declare_dram_parameter("out_pst", [2, 128, ncolp], bf16,
                                        isOutput=True)
    out_e = nc.declare_dram_parameter("out_e", [BS, D], bf16, isOutput=True)

    with tile.TileContext(nc) as tc:
        with (
            tc.tile_pool(name="const", bufs=1) as cpool,
            tc.tile_pool(name="instream", bufs=8) as in_pool,
            tc.tile_pool(name="pavgT", bufs=2) as pavgT_pool,
            tc.tile_pool(name="emb", bufs=2) as emb_pool,
            tc.tile_pool(name="wrp", bufs=3) as wr_pool,
            tc.tile_pool(name="psel", bufs=3) as psel_pool,
            tc.tile_pool(name="eT", bufs=1) as eT_pool,
            tc.tile_pool(name="cout", bufs=2) as cout_pool,
            tc.tile_pool(name="pp", bufs=2, space="PSUM") as pool_psum,
            tc.tile_pool(name="tp", bufs=2, space="PSUM") as t_psum,
            tc.tile_pool(name="pj", bufs=2, space="PSUM") as proj_psum,
            tc.tile_pool(name="cp", bufs=1, space="PSUM") as c_psum,
            tc.tile_pool(name="dram", bufs=1, space="DRAM") as dram_pool,
        ):
            wpt_sb = cpool.tile([128, 16, D], bf16, tag="wpt")
            nc.sync.dma_start(
                wpt_sb[:], wpt[:].rearrange("(c p) i -> p c i", p=128)
            )
            wet_sb = cpool.tile([128, 16, D], bf16, tag="wet")
            nc.sync.dma_start(
                wet_sb[:], wet[:].rearrange("(c p) i -> p c i", p=128)
            )
            bp_sb = cpool.tile([1, D], bf16, tag="bp")
            nc.sync.dma_start(bp_sb[:], bpp[:])
            be_sb = cpool.tile([1, D], bf16, tag="be")
            nc.sync.dma_start(be_sb[:], bee[:])
            ones16_sb = cpool.tile([128, 8], bf16, tag="ones16")
            nc.sync.dma_start(ones16_sb[:], ones16[:])
            ones1_sb = cpool.tile([1, 128], bf16, tag="ones1")
            nc.sync.dma_start(ones1_sb[:], ones1[:])
            ident_sb = cpool.tile([128, 128], bf16, tag="ident")
            nc.sync.dma_start(ident_sb[:], ident[:])

            # p_embed staged in the col-padded layout: padded column l is
            # row l.
            pe_dram = dram_pool.tile([ncolp, D], bf16)

            lidx_sb = cpool.tile([128, NB], i32, tag="lidx")
            nc.sync.dma_start(lidx_sb[:], lidx[:])

            if probe:
                # DMA-only mode: stand-in source tiles for output DMAs.
                pemb = cpool.tile([128, D], bf16, tag="pemb")
                nc.vector.memset(pemb[:], 0)
                pco = cpool.tile([128, 2, 512], bf16, tag="pco")
                nc.vector.memset(pco[:], 0)

            # ---- pool + project ----
            # Pooling matmul uses the input chunk as the stationary (lhsT)
            # operand, so the pooled sums come out already transposed:
            # psj[c, b] = sum_t in[(b,t), c].
            def stage_a_steps(src, wt_sb, b_sb, writer):
                for blk in range(NB):
                    pavgT = pavgT_pool.tile([128, 16, 128], bf16)
                    for j in range(16):
                        it = in_pool.tile([128, C], bf16, tag="instream")
                        r0 = (blk * 16 + j) * 128
                        nc.sync.dma_start(it[:], src[r0:r0 + 128, :])
                        if probe:
                            continue
                        psj = pool_psum.tile([128, 16, 8], f32)
                        for c in range(16):
                            nc.tensor.matmul(
                                psj[:, c, :],
                                lhsT=it[:, 128 * c:128 * (c + 1)],
                                rhs=ones16_sb[:],
                                start=True, stop=True,
                            )
                        nc.vector.tensor_copy(
                            pavgT[:, :, 8 * j:8 * (j + 1)], psj[:]
                        )
                        if j < 15:
                            yield
                    if probe:
                        writer(blk, pemb)
                        continue
                    pj = proj_psum.tile([128, D], f32)
                    for c in range(16):
                        nc.tensor.matmul(
                            pj[:],
                            lhsT=pavgT[:, c, :],
                            rhs=wt_sb[:, c, :],
                            start=(c == 0), stop=False,
                        )
                    nc.tensor.matmul(
                        pj[:], lhsT=ones1_sb[:], rhs=b_sb[:],
                        start=False, stop=True,
                    )
                    emb = emb_pool.tile([128, D], bf16)
                    nc.vector.tensor_copy(emb[:], pj[:])
                    writer(blk, emb)
                    yield

            def write_pe(blk, emb):
                nc.gpsimd.indirect_dma_start(
                    out=pe_dram[:],
                    out_offset=bass.IndirectOffsetOnAxis(
                        ap=lidx_sb[:, blk:blk + 1], axis=0),
                    in_=emb[:], in_offset=None,
                    bounds_check=ncolp - 1, oob_is_err=False,
                )

            def write_oute(blk, emb):
                nc.gpsimd.dma_start(out_e[blk * 128:(blk + 1) * 128, :], emb[:])

            def stage_c_steps():
                # Embeddings back in, transposed to put the embedding dim
                # on partitions: eT[j, jc, col].
                eT = eT_pool.tile([128, 2, ncolp], bf16, tag="eT")
                for gi in range(ngrp):
                    psel = psel_pool.tile([128, D], bf16)
                    nc.gpsimd.dma_start(
                        psel[:], pe_dram[128 * gi:128 * (gi + 1), :])
                    if probe:
                        continue
                    for jc in range(2):
                        tp = t_psum.tile([128, 128], bf16, tag="tp")
                        nc.tensor.transpose(
                            tp[:], psel[:, 128 * jc:128 * (jc + 1)],
                            ident_sb[:])
                        nc.vector.tensor_copy(
                            eT[:, jc, 128 * gi:128 * (gi + 1)], tp[:])
                    yield

                # Slot matmuls: 4 per slot (2 j-chunks x 2 d-chunks), W
                # stationary, 8 embedding columns moving, accumulating
                # into [128, <=512]-column PSUM chunks (d on partitions).
                for ch in range(nchunk):
                    cw = min(512, ncolp - 512 * ch)
                    cps = [c_psum.tile([128, cw], f32, name=f"cps{dc}",
                                       tag=f"cps{dc}")
                           for dc in range(2)]
                    s_lo, s_hi = ch * 64, min(nslots, (ch + 1) * 64)
                    for s4 in range(s_lo // 4, (s_hi + 3) // 4):
                        wr_sb = wr_pool.tile([128, 4, 2, D], bf16)
                        nc.scalar.dma_start(wr_sb[:], wr[s4, :, :, :, :])
                        if probe:
                            continue
                        for k in range(4):
                            s = 4 * s4 + k
                            c0 = 8 * s - 512 * ch
                            for dc in range(2):
                                for jc in range(2):
                                    nc.tensor.matmul(
                                        cps[dc][:, c0:c0 + 8],
                                        lhsT=wr_sb[:, k, jc,
                                                   128 * dc:128 * (dc + 1)],
                                        rhs=eT[:, jc, 8 * s:8 * s + 8],
                                        start=(jc == 0), stop=(jc == 1),
                                    )
                        yield
                    if probe:
                        nc.gpsimd.dma_start(
                            out_pst[:, :, 512 * ch:512 * ch + cw]
                            .rearrange("dc p w -> p dc w"),
                            pco[:, :, :cw])
                        continue
                    co = cout_pool.tile([128, 2, cw], bf16)
                    for dc in range(2):
                        nc.vector.tensor_copy(co[:, dc, :], cps[dc][:])
                    nc.gpsimd.dma_start(
                        out_pst[:, :, 512 * ch:512 * ch + cw]
                        .rearrange("dc p w -> p dc w"),
                        co[:])
                    yield

            # precondition stream first (stage C needs its embeddings),
            # then the MoE groups (self-paced on the gpsimd queue), then
            # the effect stream on the sync queue.  reps>1 re-emits the
            # whole body for slope-based exec timing (bench_pair.py).
            for _ in range(reps):
                for _ in stage_a_steps(prec, wpt_sb, bp_sb, write_pe):
                    pass
                for _ in stage_c_steps():
                    pass
                for _ in stage_a_steps(eff, wet_sb, be_sb, write_oute):
                    pass

    nc.compile()
    return nc


def _get_program(nslots, reps=1):
    key = (nslots, reps)
    if key not in _CACHE:
        _CACHE[key] = _build(nslots, reps)
    return _CACHE[key]


def _pack_slots(a_sorted):
    """Per-core slot packing: runs of equal action split into <=SW chunks."""
    per_core = []
    max_slots = 0
    for i in range(M):
        ai = a_sorted[i * BS:(i + 1) * BS]
        bounds = np.flatnonzero(np.diff(ai)) + 1
        starts = np.concatenate([[0], bounds])
        ends = np.concatenate([bounds, [BS]])
        sl = []
        for s0, s1 in zip(starts, ends):
            a = int(ai[s0])
            for cs in range(int(s0), int(s1), SW):
                sl.append((a, cs, min(cs + SW, int(s1))))
        per_core.append(sl)
        max_slots = max(max_slots, len(sl))
    return per_core, max_slots


def kernel(precondition, effect, action, Wp, bp, We, be, W_trans):
    import ml_dtypes
    from concourse.bass_utils import run_bass_kernel_spmd

    bf16 = ml_dtypes.bfloat16
    precondition = np.asarray(precondition, dtype=np.float32)
    effect = np.asarray(effect, dtype=np.float32)
    act = np.asarray(action).astype(np.int64)
    Wp = np.asarray(Wp, dtype=np.float32)
    bp = np.asarray(bp, dtype=np.float32)
    We = np.asarray(We, dtype=np.float32)
    be = np.asarray(be, dtype=np.float32)
    W_trans = np.asarray(W_trans, dtype=np.float32)

    order = np.argsort(act, kind="stable")
    a_s = act[order]

    per_core, max_slots = _pack_slots(a_s)
    nslots = max(16, ((max_slots + 15) // 16) * 16)
    nc = _get_program(nslots)
    ncolp = nslots * SW

    WpT = (np.ascontiguousarray(Wp.T) / np.float32(T)).astype(bf16)
    WeT = (np.ascontiguousarray(We.T) / np.float32(T)).astype(bf16)
    ones16 = np.zeros([128, 8], bf16)
    ones16[np.arange(128), np.arange(128) // 16] = 1.0
    ones1 = np.ones([1, 128], bf16)
    ident = np.eye(128, dtype=bf16)
    bp2 = bp.reshape(1, D).astype(bf16)
    be2 = be.reshape(1, D).astype(bf16)

    in_maps = []
    lanes_per_core = []
    for i in range(M):
        rows = order[i * BS:(i + 1) * BS]
        prec_i = np.ascontiguousarray(
            precondition[rows].reshape(BS * T, C).astype(bf16))
        eff_i = np.ascontiguousarray(
            effect[i * BS:(i + 1) * BS].reshape(BS * T, C).astype(bf16))
        sl = per_core[i]
        acts = np.zeros([nslots], np.int64)
        lane = np.zeros([BS], np.int32)   # sample -> padded column
        for r, (a, s0, s1) in enumerate(sl):
            acts[r] = a
            lane[s0:s1] = r * SW + np.arange(s1 - s0, dtype=np.int32)
        # wr[s4, jp, k, jc, d] = W_trans[acts[4*s4+k]][d_full, 128*jc+jp]
        # i.e. per-slot W^T chunked [2 jc, 128 jp, 256 d].
        wt = W_trans[acts].transpose(0, 2, 1)       # [nslots, j, d]
        wt = wt.reshape(nslots // 4, 4, 2, 128, D)  # (s4, k, jc, jp, d)
        wr = np.ascontiguousarray(
            wt.transpose(0, 3, 1, 2, 4)).astype(bf16)
        in_maps.append({
            "prec": prec_i, "eff": eff_i,
            "wpt": WpT, "wet": WeT, "bp": bp2, "be": be2,
            "ones16": ones16, "ones1": ones1, "ident": ident,
            "wr": wr, "lidx": np.ascontiguousarray(lane.reshape(NB, 128).T),
        })
        lanes_per_core.append(lane)

    global LAST_RES
    res = run_bass_kernel_spmd(nc, in_maps, list(range(M)),
                               trace=TRACE, **TRACE_KWARGS)
    LAST_RES = res

    out_p = np.empty([B, D], np.float32)
    out_e = np.empty([B, D], np.float32)
    for i in range(M):
        # out_pst[dc, p, col] -> [256, ncolp]; de-pad + unpermute.
        ot = np.asarray(res.results[i]["out_pst"], dtype=np.float32)
        ot = ot.reshape(D, ncolp)
        out_p[order[i * BS:(i + 1) * BS]] = ot[:, lanes_per_core[i]].T
        out_e[i * BS:(i + 1) * BS] = np.asarray(
            res.results[i]["out_e"], dtype=np.float32)
    return out_p, out_e


# revision 31
# speedup vs baseline: 1.0161x; 1.0161x over previous
"""ActTransNet Trainium2 kernel (8 NeuronCores, SPMD).

Computation (per reference):
    p_avg = mean_t(precondition)            [B, 2048]
    e_avg = mean_t(effect)                  [B, 2048]
    p_embed = p_avg @ Wp.T + bp             [B, 256]
    e_embed = e_avg @ We.T + be             [B, 256]   <- output 1
    p_trans = W_trans[action] @ p_embed     [B, 256]   <- output 0

Strategy (memory-bound; everything stream-shaped is bf16, accumulation
stays f32 in PSUM, outputs round to bf16 and are upcast on host):
  - Batch is argsorted by action on the host; core i gets sorted samples
    [512*i, 512*(i+1)).  Sorting makes each core's distinct-action count
    ~B_unique/8 (~125), so each needed [256,256] table row is DMA'd once.
  - Effect path is sharded in original order (no routing needed).
  - Mean-pool over T=16 runs on the TensorEngine via a block-diagonal
    ones matrix (8 samples/tile, full 128-partition contraction); the
    1/16 is folded into the host-transposed projection weights.
  - Projection: PE matmul contracting over 2048 (pooled activations are
    PE-transposed to put the contraction dim on partitions).
  - MoE stage (transposed dataflow): host packs each per-core run of
    equal-action samples into slots of <=8 samples at fixed padded
    columns [8s, 8s+8).  p-embeddings are scattered once (4 indirect
    DMAs, host-built sample->column map) into a col-padded DRAM staging
    [ncolp, 256]; groups of 128 columns are read back, PE-transposed
    into eT [128 j, 2, ncolp] (embedding dim on partitions).  Each slot
    then runs 4 matmuls (2 j-chunks x 2 d-chunks) with its
    (host-gathered, transposed) W as the stationary operand and its 8
    embedding columns as the moving operand, accumulating into
    [128 d, 512]-column PSUM chunks.  Output leaves transposed
    [2, 128, ncolp] (d on partitions); the host transposes/de-pads.
    Padding columns carry garbage end-to-end and are never read back.
"""

import numpy as np

B, T, C, D, NA = 4096, 16, 2048, 256, 1000
M = 8            # cores
BS = B // M      # 512 samples per core
NB = BS // 128   # 4 blocks of 128 samples
SW = 8           # slot width (samples per slot, fixed column window)

_CACHE = {}

# Profiling hooks for test/dev (harness just calls kernel()).
TRACE = False
TRACE_KWARGS = {}
LAST_RES = None


def _build(nslots, reps=1, probe=False):
    import concourse.bacc as bacc
    import concourse.mybir as mybir
    import concourse.tile as tile
    from concourse import bass

    f32 = mybir.dt.float32
    bf16 = mybir.dt.bfloat16
    i32 = mybir.dt.int32
    assert nslots % 16 == 0
    ncolp = nslots * SW            # padded sample-columns, multiple of 128
    ngrp = ncolp // 128            # 128-column groups
    nchunk = (ncolp + 511) // 512  # PSUM output chunks

    nc = bacc.Bacc()
    prec = nc.declare_dram_parameter("prec", [BS * T, C], bf16, isOutput=False)
    eff = nc.declare_dram_parameter("eff", [BS * T, C], bf16, isOutput=False)
    wpt = nc.declare_dram_parameter("wpt", [C, D], bf16, isOutput=False)
    wet = nc.declare_dram_parameter("wet", [C, D], bf16, isOutput=False)
    bpp = nc.declare_dram_parameter("bp", [1, D], bf16, isOutput=False)
    bee = nc.declare_dram_parameter("be", [1, D], bf16, isOutput=False)
    ones16 = nc.declare_dram_parameter("ones16", [128, 8], bf16,
                                       isOutput=False)
    ones1 = nc.declare_dram_parameter("ones1", [1, 128], bf16, isOutput=False)
    ident = nc.declare_dram_parameter("ident", [128, 128], bf16,
                                      isOutput=False)
    # per-slot transposed expert weights: wr[s4, jp, k, jc, d]
    wr = nc.declare_dram_parameter("wr", [nslots // 4, 128, 4, 2, D], bf16,
                                   isOutput=False)
    # column destination of sample (blk*128+p) in the col-padded layout
    lidx = nc.declare_dram_parameter("lidx", [128, NB], i32, isOutput=False)
    out_pst = nc.declare_dram_parameter("out_pst", [2, 128, ncolp], bf16,
                                        isOutput=True)
    out_e = nc.declare_dram_parameter("out_e", [BS, D], bf16, isOutput=True)

    with tile.TileContext(nc) as tc:
        with (
            tc.tile_pool(name="const", bufs=1) as cpool,
            tc.tile_pool(name="instream", bufs=16) as in_pool,
            tc.tile_pool(name="pavgT", bufs=2) as pavgT_pool,
            tc.tile_pool(name="emb", bufs=2) as emb_pool,
            tc.tile_pool(name="wrp", bufs=6) as wr_pool,
            tc.tile_pool(name="psel", bufs=3) as psel_pool,
            tc.tile_pool(name="eT", bufs=1) as eT_pool,
            tc.tile_pool(name="cout", bufs=2) as cout_pool,
            tc.tile_pool(name="pp", bufs=2, space="PSUM") as pool_psum,
            tc.tile_pool(name="tp", bufs=2, space="PSUM") as t_psum,
            tc.tile_pool(name="pj", bufs=2, space="PSUM") as proj_psum,
            tc.tile_pool(name="cp", bufs=1, space="PSUM") as c_psum,
            tc.tile_pool(name="dram", bufs=1, space="DRAM") as dram_pool,
        ):
            wpt_sb = cpool.tile([128, 16, D], bf16, tag="wpt")
            nc.sync.dma_start(
                wpt_sb[:], wpt[:].rearrange("(c p) i -> p c i", p=128)
            )
            wet_sb = cpool.tile([128, 16, D], bf16, tag="wet")
            nc.sync.dma_start(
                wet_sb[:], wet[:].rearrange("(c p) i -> p c i", p=128)
            )
            bp_sb = cpool.tile([1, D], bf16, tag="bp")
            nc.sync.dma_start(bp_sb[:], bpp[:])
            be_sb = cpool.tile([1, D], bf16, tag="be")
            nc.sync.dma_start(be_sb[:], bee[:])
            ones16_sb = cpool.tile([128, 8], bf16, tag="ones16")
            nc.sync.dma_start(ones16_sb[:], ones16[:])
            ones1_sb = cpool.tile([1, 128], bf16, tag="ones1")
            nc.sync.dma_start(ones1_sb[:], ones1[:])
            ident_sb = cpool.tile([128, 128], bf16, tag="ident")
            nc.sync.dma_start(ident_sb[:], ident[:])

            # p_embed staged in the col-padded layout: padded column l is
            # row l.
            pe_dram = dram_pool.tile([ncolp, D], bf16)

            lidx_sb = cpool.tile([128, NB], i32, tag="lidx")
            nc.sync.dma_start(lidx_sb[:], lidx[:])

            if probe:
                # DMA-only mode: stand-in source tiles for output DMAs.
                pemb = cpool.tile([128, D], bf16, tag="pemb")
                nc.vector.memset(pemb[:], 0)
                pco = cpool.tile([128, 2, 512], bf16, tag="pco")
                nc.vector.memset(pco[:], 0)

            # ---- pool + project ----
            # Pooling matmul uses the input chunk as the stationary (lhsT)
            # operand, so the pooled sums come out already transposed:
            # psj[c, b] = sum_t in[(b,t), c].
            def stage_a_steps(src, wt_sb, b_sb, writer):
                for blk in range(NB):
                    pavgT = pavgT_pool.tile([128, 16, 128], bf16)
                    for j in range(16):
                        it = in_pool.tile([128, C], bf16, tag="instream")
                        r0 = (blk * 16 + j) * 128
                        nc.sync.dma_start(it[:], src[r0:r0 + 128, :])
                        if probe:
                            continue
                        psj = pool_psum.tile([128, 16, 8], f32)
                        for c in range(16):
                            nc.tensor.matmul(
                                psj[:, c, :],
                                lhsT=it[:, 128 * c:128 * (c + 1)],
                                rhs=ones16_sb[:],
                                start=True, stop=True,
                            )
                        nc.vector.tensor_copy(
                            pavgT[:, :, 8 * j:8 * (j + 1)], psj[:]
                        )
                        if j < 15:
                            yield
                    if probe:
                        writer(blk, pemb)
                        continue
                    pj = proj_psum.tile([128, D], f32)
                    for c in range(16):
                        nc.tensor.matmul(
                            pj[:],
                            lhsT=pavgT[:, c, :],
                            rhs=wt_sb[:, c, :],
                            start=(c == 0), stop=False,
                        )
                    nc.tensor.matmul(
                        pj[:], lhsT=ones1_sb[:], rhs=b_sb[:],
                        start=False, stop=True,
                    )
                    emb = emb_pool.tile([128, D], bf16)
                    nc.vector.tensor_copy(emb[:], pj[:])
                    writer(blk, emb)
                    yield

            def write_pe(blk, emb):
                nc.gpsimd.indirect_dma_start(
                    out=pe_dram[:],
                    out_offset=bass.IndirectOffsetOnAxis(
                        ap=lidx_sb[:, blk:blk + 1], axis=0),
                    in_=emb[:], in_offset=None,
                    bounds_check=ncolp - 1, oob_is_err=False,
                )

            def write_oute(blk, emb):
                nc.gpsimd.dma_start(out_e[blk * 128:(blk + 1) * 128, :], emb[:])

            def stage_c_steps():
                # Embeddings back in, transposed to put the embedding dim
                # on partitions: eT[j, jc, col].
                eT = eT_pool.tile([128, 2, ncolp], bf16, tag="eT")
                for gi in range(ngrp):
                    psel = psel_pool.tile([128, D], bf16)
                    nc.gpsimd.dma_start(
                        psel[:], pe_dram[128 * gi:128 * (gi + 1), :])
                    if probe:
                        continue
                    for jc in range(2):
                        tp = t_psum.tile([128, 128], bf16, tag="tp")
                        nc.tensor.transpose(
                            tp[:], psel[:, 128 * jc:128 * (jc + 1)],
                            ident_sb[:])
                        nc.vector.tensor_copy(
                            eT[:, jc, 128 * gi:128 * (gi + 1)], tp[:])
                    yield

                # Slot matmuls: 4 per slot (2 j-chunks x 2 d-chunks), W
                # stationary, 8 embedding columns moving, accumulating
                # into [128, <=512]-column PSUM chunks (d on partitions).
                for ch in range(nchunk):
                    cw = min(512, ncolp - 512 * ch)
                    cps = [c_psum.tile([128, cw], f32, name=f"cps{dc}",
                                       tag=f"cps{dc}")
                           for dc in range(2)]
                    s_lo, s_hi = ch * 64, min(nslots, (ch + 1) * 64)
                    for s4 in range(s_lo // 4, (s_hi + 3) // 4):
                        wr_sb = wr_pool.tile([128, 4, 2, D], bf16)
                        nc.scalar.dma_start(wr_sb[:], wr[s4, :, :, :, :])
                        if probe:
                            continue
                        for k in range(4):
                            s = 4 * s4 + k
                            c0 = 8 * s - 512 * ch
                            for dc in range(2):
                                for jc in range(2):
                                    nc.tensor.matmul(
                                        cps[dc][:, c0:c0 + 8],
                                        lhsT=wr_sb[:, k, jc,
                                                   128 * dc:128 * (dc + 1)],
                                        rhs=eT[:, jc, 8 * s:8 * s + 8],
                                        start=(jc == 0), stop=(jc == 1),
                                    )
                        yield
                    if probe:
                        nc.gpsimd.dma_start(
                            out_pst[:, :, 512 * ch:512 * ch + cw]
                            .rearrange("dc p w -> p dc w"),
                            pco[:, :, :cw])
                        continue
                    co = cout_pool.tile([128, 2, cw], bf16)
                    for dc in range(2):
                        nc.vector.tensor_copy(co[:, dc, :], cps[dc][:])
                    nc.gpsimd.dma_start(
                        out_pst[:, :, 512 * ch:512 * ch + cw]
                        .rearrange("dc p w -> p dc w"),
                        co[:])
                    yield

            # precondition stream first (stage C needs its embeddings),
            # then the MoE groups (self-paced on the gpsimd queue), then
            # the effect stream on the sync queue.  reps>1 re-emits the
            # whole body for slope-based exec timing (bench_pair.py).
            for _ in range(reps):
                for _ in stage_a_steps(prec, wpt_sb, bp_sb, write_pe):
                    pass
                for _ in stage_c_steps():
                    pass
                for _ in stage_a_steps(eff, wet_sb, be_sb, write_oute):
                    pass

    nc.compile()
    return nc


def _get_program(nslots, reps=1):
    key = (nslots, reps)
    if key not in _CACHE:
        _CACHE[key] = _build(nslots, reps)
    return _CACHE[key]


def _pack_slots(a_sorted):
    """Per-core slot packing: runs of equal action split into <=SW chunks."""
    per_core = []
    max_slots = 0
    for i in range(M):
        ai = a_sorted[i * BS:(i + 1) * BS]
        bounds = np.flatnonzero(np.diff(ai)) + 1
        starts = np.concatenate([[0], bounds])
        ends = np.concatenate([bounds, [BS]])
        sl = []
        for s0, s1 in zip(starts, ends):
            a = int(ai[s0])
            for cs in range(int(s0), int(s1), SW):
                sl.append((a, cs, min(cs + SW, int(s1))))
        per_core.append(sl)
        max_slots = max(max_slots, len(sl))
    return per_core, max_slots


def kernel(precondition, effect, action, Wp, bp, We, be, W_trans):
    import ml_dtypes
    from concourse.bass_utils import run_bass_kernel_spmd

    bf16 = ml_dtypes.bfloat16
    precondition = np.asarray(precondition, dtype=np.float32)
    effect = np.asarray(effect, dtype=np.float32)
    act = np.asarray(action).astype(np.int64)
    Wp = np.asarray(Wp, dtype=np.float32)
    bp = np.asarray(bp, dtype=np.float32)
    We = np.asarray(We, dtype=np.float32)
    be = np.asarray(be, dtype=np.float32)
    W_trans = np.asarray(W_trans, dtype=np.float32)

    order = np.argsort(act, kind="stable")
    a_s = act[order]

    per_core, max_slots = _pack_slots(a_s)
    nslots = max(16, ((max_slots + 15) // 16) * 16)
    nc = _get_program(nslots)
    ncolp = nslots * SW

    WpT = (np.ascontiguousarray(Wp.T) / np.float32(T)).astype(bf16)
    WeT = (np.ascontiguousarray(We.T) / np.float32(T)).astype(bf16)
    ones16 = np.zeros([128, 8], bf16)
    ones16[np.arange(128), np.arange(128) // 16] = 1.0
    ones1 = np.ones([1, 128], bf16)
    ident = np.eye(128, dtype=bf16)
    bp2 = bp.reshape(1, D).astype(bf16)
    be2 = be.reshape(1, D).astype(bf16)

    in_maps = []
    lanes_per_core = []
    for i in range(M):
        rows = order[i * BS:(i + 1) * BS]
        prec_i = np.ascontiguousarray(
            precondition[rows].reshape(BS * T, C).astype(bf16))
        eff_i = np.ascontiguousarray(
            effect[i * BS:(i + 1) * BS].reshape(BS * T, C).astype(bf16))
        sl = per_core[i]
        acts = np.zeros([nslots], np.int64)
        lane = np.zeros([BS], np.int32)   # sample -> padded column
        for r, (a, s0, s1) in enumerate(sl):
            acts[r] = a
            lane[s0:s1] = r * SW + np.arange(s1 - s0, dtype=np.int32)
        # wr[s4, jp, k, jc, d] = W_trans[acts[4*s4+k]][d_full, 128*jc+jp]
        # i.e. per-slot W^T chunked [2 jc, 128 jp, 256 d].
        wt = W_trans[acts].transpose(0, 2, 1)       # [nslots, j, d]
        wt = wt.reshape(nslots // 4, 4, 2, 128, D)  # (s4, k, jc, jp, d)
        wr = np.ascontiguousarray(
            wt.transpose(0, 3, 1, 2, 4)).astype(bf16)
        in_maps.append({
            "prec": prec_i, "eff": eff_i,
            "wpt": WpT, "wet": WeT, "bp": bp2, "be": be2,
            "ones16": ones16, "ones1": ones1, "ident": ident,
            "wr": wr, "lidx": np.ascontiguousarray(lane.reshape(NB, 128).T),
        })
        lanes_per_core.append(lane)

    global LAST_RES
    res = run_bass_kernel_spmd(nc, in_maps, list(range(M)),
                               trace=TRACE, **TRACE_KWARGS)
    LAST_RES = res

    out_p = np.empty([B, D], np.float32)
    out_e = np.empty([B, D], np.float32)
    for i in range(M):
        # out_pst[dc, p, col] -> [256, ncolp]; de-pad + unpermute.
        ot = np.asarray(res.results[i]["out_pst"], dtype=np.float32)
        ot = ot.reshape(D, ncolp)
        out_p[order[i * BS:(i + 1) * BS]] = ot[:, lanes_per_core[i]].T
        out_e[i * BS:(i + 1) * BS] = np.asarray(
            res.results[i]["out_e"], dtype=np.float32)
    return out_p, out_e
